# revision 43
# baseline (speedup 1.0000x reference)
"""Multi-head attention kernel for Trainium2, 8 NeuronCores.

Problem: B=4, T=2048, D=1024, H=16 heads (Hd=64), fp32, full softmax
attention with key-padding mask + output projection.

Sharding: batch x head-half. Core c handles batch b=c//2 and heads
8*(c%2)..8*(c%2)+7 (feature slice of 512). Each core computes a partial
output projection (Wo row-sharded); host sums the two partials per batch.

v2 strategy (HAM-aware, ACT-bound steady state):
  - The PE clock gate (HAM) runs the array at 1.2 GHz unless it sees
    sustained activity (then 2.4 GHz). The whole kernel is emitted as ONE
    software-pipelined stream: attention steps (S matmul pair -> exp ->
    lagged PV pair) with all projection work (V proj, Q/K tiles for the
    NEXT head-pair, Wo proj) drip-fed between steps as PE filler, so the
    PE never idles and the kernel is paced by the Scalar engine's exp.
  - Attention operands are bf16 (QT/KT/pt/V/O); Q/K/V are computed from
    fp32r x/W at 11-bit precision, then rounded once to bf16.
  - Softmax denominators ride the PV matmul via a 65th 'keep' column of
    V (also folds the key-padding mask); normalization uses
    reciprocal_approx_fast + gpsimd partition_broadcast (the plain DVE
    reciprocal costs 4us per call).
  - PSUM: st(2x2 banks) + pvA/pvB(2) + pp shared by qk/V-drip/proj (2).
"""
import sys
sys.path.insert(0, "/opt/trn_rl_repo")

from contextlib import ExitStack

import numpy as np
import ml_dtypes
import concourse.bass as bass
import concourse.mybir as mybir
import concourse.tile as tile
from concourse import bacc
from concourse.bass_utils import run_bass_kernel_spmd

B, T, D, H = 4, 2048, 1024, 16
Hd = D // H          # 64
HH = H // 2          # 8 heads per core
FH = HH * Hd         # 512 features per core
P = 128
NCHUNK = T // 512    # 4 query chunks per head-pair
NDC = D // P         # 8 contraction chunks for projections
NKT = T // P         # 16 key tiles
NFT = FH // P        # 4 feature tiles (head pairs) per core

f32 = mybir.dt.float32
r32 = mybir.dt.float32r
bf16 = mybir.dt.bfloat16
ADD = mybir.AluOpType.add
MULT = mybir.AluOpType.mult
EXP = mybir.ActivationFunctionType.Exp

_cache = {}


def _round_fp32r(a):
    """Round fp32 array to fp32r (11 mantissa bits, round-nearest-even)."""
    b = np.ascontiguousarray(a, dtype=np.float32).view(np.uint32).astype(np.uint64)
    drop = 12
    half = np.uint64(1 << (drop - 1))
    lsb = (b >> np.uint64(drop)) & np.uint64(1)
    keepmask = np.uint64(~((1 << drop) - 1) & 0xFFFFFFFF)
    r = (b + half - np.uint64(1) + lsb) & keepmask
    return r.astype(np.uint32).view(np.float32).reshape(np.shape(a))


def _build(no_bias=False):
    nc = bacc.Bacc(None, target_bir_lowering=False)
    xh0 = nc.declare_dram_parameter("xh0", [P, NDC * 1024], bf16, isOutput=False)
    xh1 = nc.declare_dram_parameter("xh1", [P, NDC * 1024], bf16, isOutput=False)
    wq = nc.declare_dram_parameter("wq", [P, NDC * FH], bf16, isOutput=False)
    wk = nc.declare_dram_parameter("wk", [P, NDC * FH], bf16, isOutput=False)
    wv = nc.declare_dram_parameter("wv", [P, NDC * FH], bf16, isOutput=False)
    wo = nc.declare_dram_parameter("wo", [P, NFT * D], bf16, isOutput=False)
    bq = nc.declare_dram_parameter("bq", [FH], f32, isOutput=False)
    bk = nc.declare_dram_parameter("bk", [FH], f32, isOutput=False)
    bvr = nc.declare_dram_parameter("bvr", [P, FH], f32, isOutput=False)
    keep = nc.declare_dram_parameter("keep", [T], f32, isOutput=False)
    bo = nc.declare_dram_parameter("bo", [D], f32, isOutput=False)
    outT = nc.declare_dram_parameter("outT", [D, T], f32, isOutput=True)
    xh = [xh0, xh1]

    with tile.TileContext(nc) as tc, ExitStack() as ctx:
        const = ctx.enter_context(tc.tile_pool(name="const", bufs=1))
        qt_pool = ctx.enter_context(tc.tile_pool(name="qt", bufs=1))
        kt_pool = ctx.enter_context(tc.tile_pool(name="kt", bufs=1))
        v_pool = ctx.enter_context(tc.tile_pool(name="v", bufs=1))
        o_pool = ctx.enter_context(tc.tile_pool(name="o", bufs=1))
        ps = ctx.enter_context(tc.tile_pool(name="ps", bufs=1, space="PSUM"))
        w_pool = ctx.enter_context(tc.tile_pool(name="w", bufs=1))
        x_pool = ctx.enter_context(tc.tile_pool(name="x", bufs=1))
        pt_pool = ctx.enter_context(tc.tile_pool(name="pt", bufs=3))
        ev_pool = ctx.enter_context(tc.tile_pool(name="ev", bufs=2))
        rc_pool = ctx.enter_context(tc.tile_pool(name="rc", bufs=2))
        ot_pool = ctx.enter_context(tc.tile_pool(name="ot", bufs=2))
        vt_pool = ctx.enter_context(tc.tile_pool(name="vt", bufs=1))

        # ---- weights / x DMAs, ordered so K00/Q00 can start earliest ----
        wk_b = w_pool.tile([P, NDC, FH], bf16, tag="wkb", name="wk_b")
        wq_b = w_pool.tile([P, NDC, FH], bf16, tag="wqb", name="wq_b")
        wv_b = w_pool.tile([P, NDC, FH], bf16, tag="wvb", name="wv_b")
        xb = [x_pool.tile([P, NDC, 1024], bf16, tag=f"xb{i}", name=f"xb{i}")
              for i in range(2)]
        for i in range(4):
            cs2 = slice(i * 2 * FH, (i + 1) * 2 * FH)
            nc.sync.dma_start(out=wk_b[:, 2 * i:2 * i + 2, :], in_=wk[:, cs2])
        # x half 0 first: gates K00/Q00 and the K01 filler at step 0
        for dc in range(NDC):
            nc.sync.dma_start(out=xb[0][:, dc, 0:512],
                              in_=xh[0][:, dc * 1024:dc * 1024 + 512])
        for dc in range(NDC):
            nc.sync.dma_start(out=xb[0][:, dc, 512:1024],
                              in_=xh[0][:, dc * 1024 + 512:dc * 1024 + 1024])
        for i in range(4):
            cs2 = slice(i * 2 * FH, (i + 1) * 2 * FH)
            nc.sync.dma_start(out=wq_b[:, 2 * i:2 * i + 2, :], in_=wq[:, cs2])
        for i in range(4):
            cs2 = slice(i * 2 * FH, (i + 1) * 2 * FH)
            nc.sync.dma_start(out=wv_b[:, 2 * i:2 * i + 2, :], in_=wv[:, cs2])
        # constants needed by V chunks first, then x half 1, then the rest
        bvr_sb = const.tile([P, FH], f32, tag="bvr")
        nc.sync.dma_start(out=bvr_sb, in_=bvr[:])
        keep_sb = const.tile([P, NKT], f32, tag="keep")
        nc.sync.dma_start(out=keep_sb, in_=keep.rearrange("(c p) -> p c", p=P))
        bq_sb = const.tile([P, NFT], f32, tag="bq")
        bk_sb = const.tile([P, NFT], f32, tag="bk")
        nc.sync.dma_start(out=bq_sb, in_=bq.rearrange("(f p) -> p f", p=P))
        nc.sync.dma_start(out=bk_sb, in_=bk.rearrange("(f p) -> p f", p=P))
        zeros8 = const.tile([P, HH], f32, tag="zeros8")
        nc.vector.memset(zeros8, 0.0)
        for dc in range(NDC):
            nc.sync.dma_start(out=xb[1][:, dc, :],
                              in_=xh[1][:, dc * 1024:(dc + 1) * 1024])
        bo_sb = const.tile([P, NDC], f32, tag="bo")
        nc.sync.dma_start(out=bo_sb, in_=bo.rearrange("(d p) -> p d", p=P))

        # persistent activations (bf16)
        QT = [qt_pool.tile([P, T], bf16, tag=f"qt{i}", name=f"qt{i}")
              for i in range(NFT)]
        KT = [kt_pool.tile([P, T], bf16, tag=f"kt{i}", name=f"kt{i}")
              for i in range(NFT)]
        V = [v_pool.tile([P, HH, Hd + 1], bf16, tag=f"v{i}", name=f"v{i}")
             for i in range(NKT)]
        O = [o_pool.tile([P, T], bf16, tag=f"o{i}", name=f"o{i}")
             for i in range(NFT)]
        wo_b = w_pool.tile([P, NFT, D], bf16, tag="wob", name="wo_b")

        # ---------------- filler group emitters ----------------
        def emit_k(f, n, ptag):
            # K^T feature tile f, token chunk n (512 tokens)
            ts = slice(n * 512, (n + 1) * 512)
            fs = slice(f * P, (f + 1) * P)
            nh, off = divmod(n * 512, 1024)
            psk = ps.tile([P, 512], f32, tag=ptag, bufs=2, name="psk")
            for dc in range(NDC):
                nc.tensor.matmul(psk, wk_b[:, dc, fs],
                                 xb[nh][:, dc, off:off + 512],
                                 start=(dc == 0), stop=(dc == NDC - 1))
            nc.vector.tensor_scalar_add(KT[f][:, ts], psk, bk_sb[:, f:f + 1])

        def emit_q(f, n, ptag):
            ts = slice(n * 512, (n + 1) * 512)
            fs = slice(f * P, (f + 1) * P)
            nh, off = divmod(n * 512, 1024)
            psq = ps.tile([P, 512], f32, tag=ptag, bufs=2, name="psq")
            for dc in range(NDC):
                nc.tensor.matmul(psq, wq_b[:, dc, fs],
                                 xb[nh][:, dc, off:off + 512],
                                 start=(dc == 0), stop=(dc == NDC - 1))
            nc.vector.tensor_scalar_add(QT[f][:, ts], psq, bq_sb[:, f:f + 1])

        def emit_v(s, ptag, pbufs=2):
            # V token chunk s (128 tokens = key tile s), all 8 heads + keep col
            nh, off = divmod(s * P, 1024)
            ss = slice(off, off + P)
            psv = ps.tile([P, 512], f32, tag=ptag, bufs=pbufs, name="psv")
            for dc in range(NDC):
                nc.tensor.matmul(psv, xb[nh][:, dc, ss], wv_b[:, dc, :],
                                 start=(dc == 0), stop=(dc == NDC - 1))
            if no_bias:
                nc.vector.tensor_scalar_mul(
                    V[s][:, :, 0:Hd],
                    psv.rearrange("p (h d) -> p h d", h=HH),
                    keep_sb[:, s:s + 1])
            else:
                vtmp = vt_pool.tile([P, FH], f32, tag="vtmp", name="vtmp")
                nc.vector.tensor_tensor(vtmp, psv, bvr_sb, op=ADD)
                nc.vector.tensor_scalar_mul(
                    V[s][:, :, 0:Hd],
                    vtmp.rearrange("p (h d) -> p h d", h=HH),
                    keep_sb[:, s:s + 1])
            nc.vector.tensor_scalar_add(V[s][:, :, Hd], zeros8,
                                        keep_sb[:, s:s + 1])

        def emit_wo_dma():
            for i in range(2):
                nc.sync.dma_start(out=wo_b[:, 2 * i:2 * i + 2, :],
                                  in_=wo[:, i * 2 * D:(i + 1) * 2 * D])

        def emit_proj(j, dt_, drain=False):
            # output projection for query chunk j, D-chunk dt_. In drain
            # mode the bias-add copy runs on the (by then idle) Scalar
            # engine so the PSUM bank recycles faster.
            js = slice(j * 512, (j + 1) * 512)
            ds_ = slice(dt_ * P, (dt_ + 1) * P)
            pso = ps.tile([P, 512], f32, tag="pp", bufs=2, name="pso")
            for fc in range(NFT):
                nc.tensor.matmul(pso, wo_b[:, fc, ds_], O[fc][:, js],
                                 start=(fc == 0), stop=(fc == NFT - 1))
            ot = ot_pool.tile([P, 512], f32, tag="ot", name="ot")
            if drain:
                nc.scalar.activation(ot, pso,
                                     mybir.ActivationFunctionType.Identity,
                                     bias=bo_sb[:, dt_:dt_ + 1])
            else:
                nc.vector.tensor_scalar_add(ot, pso, bo_sb[:, dt_:dt_ + 1])
            nc.sync.dma_start(out=outT[ds_, js], in_=ot)

        # ---------------- per-step filler schedule ----------------
        # j-MAJOR order: step index = (j*4 + hp)*16 + c, 256 steps total.
        # Query chunk j's output projection spreads over ALL of era j+1
        # instead of piling onto the final head-pair.
        sched = {}

        def at(step, fn, *a, **kw):
            sched.setdefault(step, []).append((fn, a, kw))

        # era 0 / block h0: V chunks s2..15 + K(0,*) chunks drip through
        # the loop (PV lag-1 gives each V chunk execution lead); K(1,*) and
        # Q(1,0) must also land before block h1 (step 16).
        at(0, emit_v, 2, "pp")
        at(0, emit_v, 3, "pp")
        at(1, emit_k, 0, 1, "pp")     # keys 512-1023, needed at c=4
        at(1, emit_v, 4, "pp")
        at(2, emit_v, 5, "pp")
        at(3, emit_v, 6, "pp")
        at(3, emit_k, 1, 0, "pp")
        at(4, emit_v, 7, "pp")
        at(5, emit_k, 0, 2, "pp")     # needed at c=8
        at(5, emit_v, 8, "pp")
        at(6, emit_v, 9, "pp")
        at(6, emit_k, 1, 1, "pp")
        at(7, emit_v, 10, "pp")
        at(8, emit_v, 11, "pp")
        at(8, emit_q, 1, 0, "pp")
        at(9, emit_k, 0, 3, "pp")     # needed at c=12
        at(9, emit_v, 12, "pp")
        at(10, emit_v, 13, "pp")
        at(10, emit_k, 1, 2, "pp")
        at(11, emit_v, 14, "pp")
        at(12, emit_v, 15, "pp")
        at(12, emit_k, 1, 3, "pp")
        # era 0 / block h1 (16..31): f=2 tiles for block h2
        for i in range(4):
            at(17 + 3 * i, emit_k, 2, i, "pp")
        at(29, emit_q, 2, 0, "pp")
        # era 0 / block h2 (32..47): f=3 tiles for block h3
        for i in range(4):
            at(33 + 3 * i, emit_k, 3, i, "pp")
        at(45, emit_q, 3, 0, "pp")
        # era 0 / block h3 (48..63): Q(*,1) for era 1 + wo DMA
        at(48, emit_wo_dma)
        at(50, emit_q, 0, 1, "pp")
        at(54, emit_q, 1, 1, "pp")
        # eras 1..3: remaining Q(f,j) tiles ahead of their block, and
        # proj(j-1) spread across the whole era (2 groups per block).
        for j in range(1, NCHUNK):
            e = 4 * j * 16
            if j >= 1:
                at(e + 8, emit_q, 2, j, "pp")       # by block (j,2)
                at(e + 24, emit_q, 3, j, "pp")      # by block (j,3)
            if j < NCHUNK - 1:
                at(e + 38, emit_q, 0, j + 1, "pp")  # by era j+1
                at(e + 46, emit_q, 1, j + 1, "pp")
            for g, dt_ in enumerate(range(NDC)):
                at(e + 8 + g * 6, emit_proj, j - 1, dt_)

        # ---------------- phase A: startup projections ----------------
        with nc.named_scope("phaseA"):
            # HAM warm-up: the PE clock gate needs ~3.4us of sustained
            # activity to reach 2.4 GHz. Dummy matmuls on a constant tile
            # keep the PE busy through the initial DMA wait so the real
            # projections start at full clock.
            warm_in = const.tile([P, 512], bf16, tag="warm")
            nc.vector.memset(warm_in, 0.0)
            for _ in range(40):
                wst = ps.tile([P, 512], f32, tag="st", bufs=2, name="wst")
                nc.tensor.matmul(wst, warm_in[:, 0:128], warm_in,
                                 start=True, stop=True)
            emit_k(0, 0, "pp")
            emit_q(0, 0, "pp")
            for s in range(2):
                emit_v(s, "pva" if s % 2 == 0 else "pvb", 1)

        # ---------------- main pipelined attention loop ----------------
        def emit_s_exp(hp, j, c):
            js = slice(j * 512, (j + 1) * 512)
            cs = slice(c * P, (c + 1) * P)
            st = ps.tile([P, 1024], f32, tag="st", bufs=2, name="st")
            nc.tensor.matmul(st[:, 0:512], KT[hp][0:64, cs], QT[hp][0:64, js],
                             start=True, stop=True, tile_position=(0, 0))
            nc.tensor.matmul(st[:, 512:1024], KT[hp][64:128, cs],
                             QT[hp][64:128, js],
                             start=True, stop=True, tile_position=(64, 0))
            pt = pt_pool.tile([P, 1024], bf16, tag="pt", name="pt")
            nc.scalar.activation(pt, st, EXP)
            return pt

        def emit_pv(hp, c, pt, pvA, pvB):
            nc.tensor.matmul(pvA[0:Hd + 1, :], V[c][:, 2 * hp, :],
                             pt[:, 0:512],
                             start=(c == 0), stop=(c == NKT - 1))
            nc.tensor.matmul(pvB[0:Hd + 1, :], V[c][:, 2 * hp + 1, :],
                             pt[:, 512:1024],
                             start=(c == 0), stop=(c == NKT - 1))

        def emit_normalize(hp, j, pvA, pvB, drain=False):
            # ev copies release both PSUM banks first; then the two softmax
            # denominators are gathered into one [33,512] tile (partition
            # offsets must be 32-aligned) so a single free-size-bound DVE
            # reciprocal covers both heads. In drain mode no later PV needs
            # the banks, so the den copies go first and the reciprocal
            # starts as early as possible.
            js = slice(j * 512, (j + 1) * 512)
            den2 = rc_pool.tile([33, 512], f32, tag="den2", bufs=1,
                                name="den2")
            rec2 = rc_pool.tile([33, 512], f32, tag="rec2", bufs=1,
                                name="rec2")
            recB = rc_pool.tile([1, 512], f32, tag="recB", bufs=1,
                                name="recB")
            nc.vector.memset(den2, 1.0)
            evs = []

            def dens(srcA, srcB):
                nc.vector.tensor_copy(den2[0:1, :], srcA)
                nc.vector.tensor_copy(den2[32:33, :], srcB)
                nc.vector.reciprocal(rec2, den2)
                nc.vector.tensor_copy(recB, rec2[32:33, :])

            if drain:
                dens(pvA[Hd:Hd + 1, :], pvB[Hd:Hd + 1, :])
            for pv in (pvA, pvB):
                ev = ev_pool.tile([Hd + 1, 512], f32, tag="ev", name="ev")
                nc.vector.tensor_copy(ev, pv[0:Hd + 1, :])
                evs.append(ev)
            if not drain:
                dens(evs[0][Hd:Hd + 1, :], evs[1][Hd:Hd + 1, :])
            for h, (ev, rsrc) in enumerate(zip(evs, (rec2, recB))):
                rrep = rc_pool.tile([Hd, 512], f32, tag=f"rrep{h}", bufs=1,
                                    name="rrep")
                nc.gpsimd.partition_broadcast(rrep, rsrc[0:1, :])
                rows = slice(h * Hd, (h + 1) * Hd)
                nc.vector.tensor_tensor(O[hp][rows, js], ev[0:Hd, :], rrep,
                                        op=MULT)

        with nc.named_scope("attn"):
            # prev = (hp, j, c, pt, pvA, pvB): PV pair lagging one step so
            # the PE never waits on exp; the normalize for a query chunk is
            # emitted immediately after its final (c==NKT-1) PV pair.
            prev = None
            for j in range(NCHUNK):
                for hp in range(NFT):
                    pvA = ps.tile([P, 512], f32, tag="pva", bufs=1,
                                  name="pva")
                    pvB = ps.tile([P, 512], f32, tag="pvb", bufs=1,
                                  name="pvb")
                    for c in range(NKT):
                        step = (j * NFT + hp) * NKT + c
                        pt = emit_s_exp(hp, j, c)
                        boundary = prev is not None and prev[2] == NKT - 1
                        if boundary:
                            # final PV + bank-releasing normalize go ahead
                            # of the step's fillers in every engine FIFO
                            ph399, pj, pc, ppt, ppvA, ppvB = prev
                            emit_pv(ph399, pc, ppt, ppvA, ppvB)
                            emit_normalize(ph399, pj, ppvA, ppvB)
                        for fn, a, kw in sched.get(step, ()):
                            fn(*a, **kw)
                        if prev is not None and not boundary:
                            ph399, pj, pc, ppt, ppvA, ppvB = prev
                            emit_pv(ph399, pc, ppt, ppvA, ppvB)
                        prev = (hp, j, c, pt, pvA, pvB)
            # drain: last PV, last normalize, last proj chunk
            ph399, pj, pc, ppt, ppvA, ppvB = prev
            emit_pv(ph399, pc, ppt, ppvA, ppvB)
            emit_normalize(ph399, pj, ppvA, ppvB, drain=True)
            for dt_ in range(NDC):
                emit_proj(3, dt_, drain=(dt_ % 2 == 1))

    nc.compile()
    return nc


def _get_nc(no_bias=False):
    key = ("nc", no_bias)
    if key not in _cache:
        _cache[key] = _build(no_bias)
    return _cache[key]


def kernel(x, mask, Wq, bq, Wk, bk, Wv, bv, Wo, bo):
    x = np.asarray(x, dtype=np.float32)
    mask = np.asarray(mask)
    Wq = np.asarray(Wq, dtype=np.float32)
    bq = np.asarray(bq, dtype=np.float32)
    Wk = np.asarray(Wk, dtype=np.float32)
    bk = np.asarray(bk, dtype=np.float32)
    Wv = np.asarray(Wv, dtype=np.float32)
    bv = np.asarray(bv, dtype=np.float32)
    Wo = np.asarray(Wo, dtype=np.float32)
    bo = np.asarray(bo, dtype=np.float32)

    scale = np.float32(Hd) ** -0.5
    no_bias = not (bq.any() or bk.any() or bv.any() or bo.any())
    nc = _get_nc(no_bias)

    def pack_w(w):
        # [D, FH] -> [128, (dc f)]: partition p line = concat over dc of
        # w[dc*128+p, :]
        return np.ascontiguousarray(
            w.astype(ml_dtypes.bfloat16).reshape(NDC, P, FH)
            .transpose(1, 0, 2).reshape(P, NDC * FH))

    in_maps = []
    for core in range(8):
        b, s = core // 2, core % 2
        sl = slice(s * FH, (s + 1) * FH)
        xr = x[b].T.astype(ml_dtypes.bfloat16).reshape(NDC, P, T)
        wo_p = (Wo[sl, :].astype(ml_dtypes.bfloat16)
                .reshape(NFT, P, D).transpose(1, 0, 2).reshape(P, NFT * D))
        m = {
            "xh0": np.ascontiguousarray(
                xr[:, :, 0:1024].transpose(1, 0, 2).reshape(P, NDC * 1024)),
            "xh1": np.ascontiguousarray(
                xr[:, :, 1024:2048].transpose(1, 0, 2).reshape(P, NDC * 1024)),
            "wq": pack_w(Wq[:, sl] * scale),
            "wk": pack_w(Wk[:, sl]),
            "wv": pack_w(Wv[:, sl]),
            "wo": np.ascontiguousarray(wo_p),
            "bq": np.ascontiguousarray(bq[sl] * scale),
            "bk": np.ascontiguousarray(bk[sl]),
            "bvr": np.ascontiguousarray(np.broadcast_to(bv[sl], (P, FH))),
            "keep": (1.0 - mask[b].astype(np.float32)),
            "bo": bo if s == 0 else np.zeros_like(bo),
        }
        in_maps.append(m)

    global _last_in_maps
    _last_in_maps = in_maps
    res = run_bass_kernel_spmd(nc, in_maps, list(range(8)))
    out = np.empty((B, T, D), dtype=np.float32)
    for b in range(B):
        acc = res.results[2 * b]["outT"] + res.results[2 * b + 1]["outT"]
        out[b] = acc.T
    return out


# revision 46
# speedup vs baseline: 1.0047x; 1.0047x over previous
"""Multi-head attention kernel for Trainium2, 8 NeuronCores.

Problem: B=4, T=2048, D=1024, H=16 heads (Hd=64), fp32, full softmax
attention with key-padding mask + output projection.

Sharding: batch x head-half. Core c handles batch b=c//2 and heads
8*(c%2)..8*(c%2)+7 (feature slice of 512). Each core computes a partial
output projection (Wo row-sharded); host sums the two partials per batch.

v2 strategy (HAM-aware, ACT-bound steady state):
  - The PE clock gate (HAM) runs the array at 1.2 GHz unless it sees
    sustained activity (then 2.4 GHz). The whole kernel is emitted as ONE
    software-pipelined stream: attention steps (S matmul pair -> exp ->
    lagged PV pair) with all projection work (V proj, Q/K tiles for the
    NEXT head-pair, Wo proj) drip-fed between steps as PE filler, so the
    PE never idles and the kernel is paced by the Scalar engine's exp.
  - Attention operands are bf16 (QT/KT/pt/V/O); Q/K/V are computed from
    fp32r x/W at 11-bit precision, then rounded once to bf16.
  - Softmax denominators ride the PV matmul via a 65th 'keep' column of
    V (also folds the key-padding mask); normalization uses
    reciprocal_approx_fast + gpsimd partition_broadcast (the plain DVE
    reciprocal costs 4us per call).
  - PSUM: st(2x2 banks) + pvA/pvB(2) + pp shared by qk/V-drip/proj (2).
"""
import sys
sys.path.insert(0, "/opt/trn_rl_repo")

from contextlib import ExitStack

import numpy as np
import ml_dtypes
import concourse.bass as bass
import concourse.mybir as mybir
import concourse.tile as tile
from concourse import bacc
from concourse.bass_utils import run_bass_kernel_spmd

B, T, D, H = 4, 2048, 1024, 16
Hd = D // H          # 64
HH = H // 2          # 8 heads per core
FH = HH * Hd         # 512 features per core
P = 128
NCHUNK = T // 512    # 4 query chunks per head-pair
NDC = D // P         # 8 contraction chunks for projections
NKT = T // P         # 16 key tiles
NFT = FH // P        # 4 feature tiles (head pairs) per core

f32 = mybir.dt.float32
r32 = mybir.dt.float32r
bf16 = mybir.dt.bfloat16
ADD = mybir.AluOpType.add
MULT = mybir.AluOpType.mult
EXP = mybir.ActivationFunctionType.Exp

_cache = {}


def _round_fp32r(a):
    """Round fp32 array to fp32r (11 mantissa bits, round-nearest-even)."""
    b = np.ascontiguousarray(a, dtype=np.float32).view(np.uint32).astype(np.uint64)
    drop = 12
    half = np.uint64(1 << (drop - 1))
    lsb = (b >> np.uint64(drop)) & np.uint64(1)
    keepmask = np.uint64(~((1 << drop) - 1) & 0xFFFFFFFF)
    r = (b + half - np.uint64(1) + lsb) & keepmask
    return r.astype(np.uint32).view(np.float32).reshape(np.shape(a))


def _build(no_bias=False):
    nc = bacc.Bacc(None, target_bir_lowering=False)
    xh0 = nc.declare_dram_parameter("xh0", [P, NDC * 1024], bf16, isOutput=False)
    xh1 = nc.declare_dram_parameter("xh1", [P, NDC * 1024], bf16, isOutput=False)
    wq = nc.declare_dram_parameter("wq", [P, NDC * FH], bf16, isOutput=False)
    wk = nc.declare_dram_parameter("wk", [P, NDC * FH], bf16, isOutput=False)
    wv = nc.declare_dram_parameter("wv", [P, NDC * FH], bf16, isOutput=False)
    wo = nc.declare_dram_parameter("wo", [P, NFT * D], bf16, isOutput=False)
    bq = nc.declare_dram_parameter("bq", [FH], f32, isOutput=False)
    bk = nc.declare_dram_parameter("bk", [FH], f32, isOutput=False)
    bvr = nc.declare_dram_parameter("bvr", [P, FH], f32, isOutput=False)
    keep = nc.declare_dram_parameter("keep", [T], f32, isOutput=False)
    bo = nc.declare_dram_parameter("bo", [D], f32, isOutput=False)
    outT = nc.declare_dram_parameter("outT", [D, T], f32, isOutput=True)
    xh = [xh0, xh1]

    with tile.TileContext(nc) as tc, ExitStack() as ctx:
        const = ctx.enter_context(tc.tile_pool(name="const", bufs=1))
        qt_pool = ctx.enter_context(tc.tile_pool(name="qt", bufs=1))
        kt_pool = ctx.enter_context(tc.tile_pool(name="kt", bufs=1))
        v_pool = ctx.enter_context(tc.tile_pool(name="v", bufs=1))
        o_pool = ctx.enter_context(tc.tile_pool(name="o", bufs=1))
        ps = ctx.enter_context(tc.tile_pool(name="ps", bufs=1, space="PSUM"))
        w_pool = ctx.enter_context(tc.tile_pool(name="w", bufs=1))
        x_pool = ctx.enter_context(tc.tile_pool(name="x", bufs=1))
        pt_pool = ctx.enter_context(tc.tile_pool(name="pt", bufs=3))
        ev_pool = ctx.enter_context(tc.tile_pool(name="ev", bufs=2))
        rc_pool = ctx.enter_context(tc.tile_pool(name="rc", bufs=2))
        ot_pool = ctx.enter_context(tc.tile_pool(name="ot", bufs=2))
        vt_pool = ctx.enter_context(tc.tile_pool(name="vt", bufs=1))

        # ---- weights / x DMAs, ordered so K00/Q00 can start earliest ----
        wk_b = w_pool.tile([P, NDC, FH], bf16, tag="wkb", name="wk_b")
        wq_b = w_pool.tile([P, NDC, FH], bf16, tag="wqb", name="wq_b")
        wv_b = w_pool.tile([P, NDC, FH], bf16, tag="wvb", name="wv_b")
        xb = [x_pool.tile([P, NDC, 1024], bf16, tag=f"xb{i}", name=f"xb{i}")
              for i in range(2)]
        for i in range(4):
            cs2 = slice(i * 2 * FH, (i + 1) * 2 * FH)
            nc.sync.dma_start(out=wk_b[:, 2 * i:2 * i + 2, :], in_=wk[:, cs2])
        # x half 0 first: gates K00/Q00 and the K01 filler at step 0
        for dc in range(NDC):
            nc.sync.dma_start(out=xb[0][:, dc, 0:512],
                              in_=xh[0][:, dc * 1024:dc * 1024 + 512])
        for dc in range(NDC):
            nc.sync.dma_start(out=xb[0][:, dc, 512:1024],
                              in_=xh[0][:, dc * 1024 + 512:dc * 1024 + 1024])
        for i in range(4):
            cs2 = slice(i * 2 * FH, (i + 1) * 2 * FH)
            nc.sync.dma_start(out=wq_b[:, 2 * i:2 * i + 2, :], in_=wq[:, cs2])
        for i in range(4):
            cs2 = slice(i * 2 * FH, (i + 1) * 2 * FH)
            nc.sync.dma_start(out=wv_b[:, 2 * i:2 * i + 2, :], in_=wv[:, cs2])
        # constants needed by V chunks first, then x half 1, then the rest
        bvr_sb = const.tile([P, FH], f32, tag="bvr")
        nc.sync.dma_start(out=bvr_sb, in_=bvr[:])
        keep_sb = const.tile([P, NKT], f32, tag="keep")
        nc.sync.dma_start(out=keep_sb, in_=keep.rearrange("(c p) -> p c", p=P))
        bq_sb = const.tile([P, NFT], f32, tag="bq")
        bk_sb = const.tile([P, NFT], f32, tag="bk")
        nc.sync.dma_start(out=bq_sb, in_=bq.rearrange("(f p) -> p f", p=P))
        nc.sync.dma_start(out=bk_sb, in_=bk.rearrange("(f p) -> p f", p=P))
        zeros8 = const.tile([P, HH], f32, tag="zeros8")
        nc.vector.memset(zeros8, 0.0)
        for dc in range(NDC):
            nc.sync.dma_start(out=xb[1][:, dc, :],
                              in_=xh[1][:, dc * 1024:(dc + 1) * 1024])
        bo_sb = const.tile([P, NDC], f32, tag="bo")
        nc.sync.dma_start(out=bo_sb, in_=bo.rearrange("(d p) -> p d", p=P))

        # persistent activations (bf16)
        QT = [qt_pool.tile([P, T], bf16, tag=f"qt{i}", name=f"qt{i}")
              for i in range(NFT)]
        KT = [kt_pool.tile([P, T], bf16, tag=f"kt{i}", name=f"kt{i}")
              for i in range(NFT)]
        V = [v_pool.tile([P, HH, Hd + 1], bf16, tag=f"v{i}", name=f"v{i}")
             for i in range(NKT)]
        O = [o_pool.tile([P, T], bf16, tag=f"o{i}", name=f"o{i}")
             for i in range(NFT)]
        wo_b = w_pool.tile([P, NFT, D], bf16, tag="wob", name="wo_b")

        # ---------------- filler group emitters ----------------
        def emit_k(f, n, ptag):
            # K^T feature tile f, token chunk n (512 tokens)
            ts = slice(n * 512, (n + 1) * 512)
            fs = slice(f * P, (f + 1) * P)
            nh, off = divmod(n * 512, 1024)
            psk = ps.tile([P, 512], f32, tag=ptag, bufs=2, name="psk")
            for dc in range(NDC):
                nc.tensor.matmul(psk, wk_b[:, dc, fs],
                                 xb[nh][:, dc, off:off + 512],
                                 start=(dc == 0), stop=(dc == NDC - 1))
            nc.vector.tensor_scalar_add(KT[f][:, ts], psk, bk_sb[:, f:f + 1])

        def emit_q(f, n, ptag):
            ts = slice(n * 512, (n + 1) * 512)
            fs = slice(f * P, (f + 1) * P)
            nh, off = divmod(n * 512, 1024)
            psq = ps.tile([P, 512], f32, tag=ptag, bufs=2, name="psq")
            for dc in range(NDC):
                nc.tensor.matmul(psq, wq_b[:, dc, fs],
                                 xb[nh][:, dc, off:off + 512],
                                 start=(dc == 0), stop=(dc == NDC - 1))
            nc.vector.tensor_scalar_add(QT[f][:, ts], psq, bq_sb[:, f:f + 1])

        def emit_v(s, ptag, pbufs=2):
            # V token chunk s (128 tokens = key tile s), all 8 heads + keep col
            nh, off = divmod(s * P, 1024)
            ss = slice(off, off + P)
            psv = ps.tile([P, 512], f32, tag=ptag, bufs=pbufs, name="psv")
            for dc in range(NDC):
                nc.tensor.matmul(psv, xb[nh][:, dc, ss], wv_b[:, dc, :],
                                 start=(dc == 0), stop=(dc == NDC - 1))
            if no_bias:
                nc.vector.tensor_scalar_mul(
                    V[s][:, :, 0:Hd],
                    psv.rearrange("p (h d) -> p h d", h=HH),
                    keep_sb[:, s:s + 1])
            else:
                vtmp = vt_pool.tile([P, FH], f32, tag="vtmp", name="vtmp")
                nc.vector.tensor_tensor(vtmp, psv, bvr_sb, op=ADD)
                nc.vector.tensor_scalar_mul(
                    V[s][:, :, 0:Hd],
                    vtmp.rearrange("p (h d) -> p h d", h=HH),
                    keep_sb[:, s:s + 1])
            nc.vector.tensor_scalar_add(V[s][:, :, Hd], zeros8,
                                        keep_sb[:, s:s + 1])

        def emit_wo_dma():
            for i in range(2):
                nc.sync.dma_start(out=wo_b[:, 2 * i:2 * i + 2, :],
                                  in_=wo[:, i * 2 * D:(i + 1) * 2 * D])

        def emit_proj(j, dt_, drain=False):
            # output projection for query chunk j, D-chunk dt_. In drain
            # mode the bias-add copy runs on the (by then idle) Scalar
            # engine so the PSUM bank recycles faster.
            js = slice(j * 512, (j + 1) * 512)
            ds_ = slice(dt_ * P, (dt_ + 1) * P)
            pso = ps.tile([P, 512], f32, tag="pp", bufs=2, name="pso")
            for fc in range(NFT):
                nc.tensor.matmul(pso, wo_b[:, fc, ds_], O[fc][:, js],
                                 start=(fc == 0), stop=(fc == NFT - 1))
            ot = ot_pool.tile([P, 512], f32, tag="ot", name="ot")
            if drain:
                nc.scalar.activation(ot, pso,
                                     mybir.ActivationFunctionType.Identity,
                                     bias=bo_sb[:, dt_:dt_ + 1])
            else:
                nc.vector.tensor_scalar_add(ot, pso, bo_sb[:, dt_:dt_ + 1])
            nc.sync.dma_start(out=outT[ds_, js], in_=ot)

        # ---------------- per-step filler schedule ----------------
        # j-MAJOR order: step index = (j*4 + hp)*16 + c, 256 steps total.
        # Query chunk j's output projection spreads over ALL of era j+1
        # instead of piling onto the final head-pair.
        sched = {}

        def at(step, fn, *a, **kw):
            sched.setdefault(step, []).append((fn, a, kw))

        # era 0 / block h0: V chunks s2..15 + K(0,*) chunks drip through
        # the loop (PV lag-1 gives each V chunk execution lead); K(1,*) and
        # Q(1,0) must also land before block h1 (step 16).
        at(0, emit_v, 2, "pp")
        at(0, emit_v, 3, "pp")
        at(1, emit_k, 0, 1, "pp")     # keys 512-1023, needed at c=4
        at(1, emit_v, 4, "pp")
        at(2, emit_v, 5, "pp")
        at(3, emit_v, 6, "pp")
        at(3, emit_k, 1, 0, "pp")
        at(4, emit_v, 7, "pp")
        at(5, emit_k, 0, 2, "pp")     # needed at c=8
        at(5, emit_v, 8, "pp")
        at(6, emit_v, 9, "pp")
        at(6, emit_k, 1, 1, "pp")
        at(7, emit_v, 10, "pp")
        at(8, emit_v, 11, "pp")
        at(8, emit_q, 1, 0, "pp")
        at(9, emit_k, 0, 3, "pp")     # needed at c=12
        at(9, emit_v, 12, "pp")
        at(10, emit_v, 13, "pp")
        at(10, emit_k, 1, 2, "pp")
        at(11, emit_v, 14, "pp")
        at(12, emit_v, 15, "pp")
        at(12, emit_k, 1, 3, "pp")
        # era 0 / block h1 (16..31): f=2 tiles for block h2
        for i in range(4):
            at(17 + 3 * i, emit_k, 2, i, "pp")
        at(29, emit_q, 2, 0, "pp")
        # era 0 / block h2 (32..47): f=3 tiles for block h3
        for i in range(4):
            at(33 + 3 * i, emit_k, 3, i, "pp")
        at(45, emit_q, 3, 0, "pp")
        # era 0 / block h3 (48..63): Q(*,1) for era 1 + wo DMA
        at(48, emit_wo_dma)
        at(50, emit_q, 0, 1, "pp")
        at(54, emit_q, 1, 1, "pp")
        # eras 1..3: remaining Q(f,j) tiles ahead of their block, and
        # proj(j-1) spread across the whole era (2 groups per block).
        for j in range(1, NCHUNK):
            e = 4 * j * 16
            if j >= 1:
                at(e + 8, emit_q, 2, j, "pp")       # by block (j,2)
                at(e + 24, emit_q, 3, j, "pp")      # by block (j,3)
            if j < NCHUNK - 1:
                at(e + 38, emit_q, 0, j + 1, "pp")  # by era j+1
                at(e + 46, emit_q, 1, j + 1, "pp")
            for g, dt_ in enumerate(range(NDC)):
                at(e + 8 + g * 6, emit_proj, j - 1, dt_)

        # ---------------- phase A: startup projections ----------------
        with nc.named_scope("phaseA"):
            # HAM warm-up: the PE clock gate needs ~3.4us of sustained
            # activity to reach 2.4 GHz. Dummy matmuls on a constant tile
            # keep the PE busy through the initial DMA wait so the real
            # projections start at full clock.
            warm_in = const.tile([P, 512], bf16, tag="warm")
            nc.vector.memset(warm_in, 0.0)
            for _ in range(24):
                wst = ps.tile([P, 512], f32, tag="st", bufs=2, name="wst")
                nc.tensor.matmul(wst, warm_in[:, 0:128], warm_in,
                                 start=True, stop=True)
            emit_k(0, 0, "pp")
            emit_q(0, 0, "pp")
            for s in range(2):
                emit_v(s, "pva" if s % 2 == 0 else "pvb", 1)

        # ---------------- main pipelined attention loop ----------------
        def emit_s_exp(hp, j, c):
            js = slice(j * 512, (j + 1) * 512)
            cs = slice(c * P, (c + 1) * P)
            st = ps.tile([P, 1024], f32, tag="st", bufs=2, name="st")
            nc.tensor.matmul(st[:, 0:512], KT[hp][0:64, cs], QT[hp][0:64, js],
                             start=True, stop=True, tile_position=(0, 0))
            nc.tensor.matmul(st[:, 512:1024], KT[hp][64:128, cs],
                             QT[hp][64:128, js],
                             start=True, stop=True, tile_position=(64, 0))
            pt = pt_pool.tile([P, 1024], bf16, tag="pt", name="pt")
            nc.scalar.activation(pt, st, EXP)
            return pt

        def emit_pv(hp, c, pt, pvA, pvB):
            nc.tensor.matmul(pvA[0:Hd + 1, :], V[c][:, 2 * hp, :],
                             pt[:, 0:512],
                             start=(c == 0), stop=(c == NKT - 1))
            nc.tensor.matmul(pvB[0:Hd + 1, :], V[c][:, 2 * hp + 1, :],
                             pt[:, 512:1024],
                             start=(c == 0), stop=(c == NKT - 1))

        def emit_normalize(hp, j, pvA, pvB, drain=False):
            # ev copies release both PSUM banks first; then the two softmax
            # denominators are gathered into one [33,512] tile (partition
            # offsets must be 32-aligned) so a single free-size-bound DVE
            # reciprocal covers both heads. In drain mode no later PV needs
            # the banks, so the den copies go first and the reciprocal
            # starts as early as possible.
            js = slice(j * 512, (j + 1) * 512)
            den2 = rc_pool.tile([33, 512], f32, tag="den2", bufs=1,
                                name="den2")
            rec2 = rc_pool.tile([33, 512], f32, tag="rec2", bufs=1,
                                name="rec2")
            recB = rc_pool.tile([1, 512], f32, tag="recB", bufs=1,
                                name="recB")
            nc.vector.memset(den2, 1.0)
            evs = []

            def dens(srcA, srcB):
                nc.vector.tensor_copy(den2[0:1, :], srcA)
                nc.vector.tensor_copy(den2[32:33, :], srcB)
                nc.vector.reciprocal(rec2, den2)
                nc.vector.tensor_copy(recB, rec2[32:33, :])

            if drain:
                dens(pvA[Hd:Hd + 1, :], pvB[Hd:Hd + 1, :])
            for pv in (pvA, pvB):
                ev = ev_pool.tile([Hd + 1, 512], f32, tag="ev", name="ev")
                nc.vector.tensor_copy(ev, pv[0:Hd + 1, :])
                evs.append(ev)
            if not drain:
                dens(evs[0][Hd:Hd + 1, :], evs[1][Hd:Hd + 1, :])
            for h, (ev, rsrc) in enumerate(zip(evs, (rec2, recB))):
                rrep = rc_pool.tile([Hd, 512], f32, tag=f"rrep{h}", bufs=1,
                                    name="rrep")
                nc.gpsimd.partition_broadcast(rrep, rsrc[0:1, :])
                rows = slice(h * Hd, (h + 1) * Hd)
                nc.vector.tensor_tensor(O[hp][rows, js], ev[0:Hd, :], rrep,
                                        op=MULT)

        with nc.named_scope("attn"):
            # prev = (hp, j, c, pt, pvA, pvB): PV pair lagging one step so
            # the PE never waits on exp; the normalize for a query chunk is
            # emitted immediately after its final (c==NKT-1) PV pair.
            # pending holds un-emitted PV work: normally one step of lag;
            # around block boundaries it grows to two so the PE FIFO never
            # blocks on the PSUM-bank-releasing ev copies.
            pending = []
            for j in range(NCHUNK):
                for hp in range(NFT):
                    pvA = ps.tile([P, 512], f32, tag="pva", bufs=1,
                                  name="pva")
                    pvB = ps.tile([P, 512], f32, tag="pvb", bufs=1,
                                  name="pvb")
                    for c in range(NKT):
                        step = (j * NFT + hp) * NKT + c
                        pt = emit_s_exp(hp, j, c)
                        if pending and pending[0][2] == NKT - 1:
                            # final PV + bank-releasing normalize ahead of
                            # the step's fillers in every engine FIFO
                            ph_, pj, pc, ppt, ppvA, ppvB = pending.pop(0)
                            emit_pv(ph_, pc, ppt, ppvA, ppvB)
                            emit_normalize(ph_, pj, ppvA, ppvB)
                        for fn, a, kw in sched.get(step, ()):
                            fn(*a, **kw)
                        if c != 1:  # at c==1 the lag grows to 2 so the PE
                            npop = 2 if c == 2 else 1  # FIFO never blocks
                            for _ in range(min(npop, len(pending))):
                                ph_, pj, pc, ppt, ppvA, ppvB = \
                                    pending.pop(0)
                                emit_pv(ph_, pc, ppt, ppvA, ppvB)
                        pending.append((hp, j, c, pt, pvA, pvB))
            # drain: last PV, last normalize, last proj chunk
            ph_, pj, pc, ppt, ppvA, ppvB = pending.pop(0)
            emit_pv(ph_, pc, ppt, ppvA, ppvB)
            emit_normalize(ph_, pj, ppvA, ppvB, drain=True)
            for dt_ in range(NDC):
                emit_proj(3, dt_, drain=(dt_ % 2 == 1))

    nc.compile()
    return nc


def _get_nc(no_bias=False):
    key = ("nc", no_bias)
    if key not in _cache:
        _cache[key] = _build(no_bias)
    return _cache[key]


def kernel(x, mask, Wq, bq, Wk, bk, Wv, bv, Wo, bo):
    x = np.asarray(x, dtype=np.float32)
    mask = np.asarray(mask)
    Wq = np.asarray(Wq, dtype=np.float32)
    bq = np.asarray(bq, dtype=np.float32)
    Wk = np.asarray(Wk, dtype=np.float32)
    bk = np.asarray(bk, dtype=np.float32)
    Wv = np.asarray(Wv, dtype=np.float32)
    bv = np.asarray(bv, dtype=np.float32)
    Wo = np.asarray(Wo, dtype=np.float32)
    bo = np.asarray(bo, dtype=np.float32)

    scale = np.float32(Hd) ** -0.5
    no_bias = not (bq.any() or bk.any() or bv.any() or bo.any())
    nc = _get_nc(no_bias)

    def pack_w(w):
        # [D, FH] -> [128, (dc f)]: partition p line = concat over dc of
        # w[dc*128+p, :]
        return np.ascontiguousarray(
            w.astype(ml_dtypes.bfloat16).reshape(NDC, P, FH)
            .transpose(1, 0, 2).reshape(P, NDC * FH))

    in_maps = []
    for core in range(8):
        b, s = core // 2, core % 2
        sl = slice(s * FH, (s + 1) * FH)
        xr = x[b].T.astype(ml_dtypes.bfloat16).reshape(NDC, P, T)
        wo_p = (Wo[sl, :].astype(ml_dtypes.bfloat16)
                .reshape(NFT, P, D).transpose(1, 0, 2).reshape(P, NFT * D))
        m = {
            "xh0": np.ascontiguousarray(
                xr[:, :, 0:1024].transpose(1, 0, 2).reshape(P, NDC * 1024)),
            "xh1": np.ascontiguousarray(
                xr[:, :, 1024:2048].transpose(1, 0, 2).reshape(P, NDC * 1024)),
            "wq": pack_w(Wq[:, sl] * scale),
            "wk": pack_w(Wk[:, sl]),
            "wv": pack_w(Wv[:, sl]),
            "wo": np.ascontiguousarray(wo_p),
            "bq": np.ascontiguousarray(bq[sl] * scale),
            "bk": np.ascontiguousarray(bk[sl]),
            "bvr": np.ascontiguousarray(np.broadcast_to(bv[sl], (P, FH))),
            "keep": (1.0 - mask[b].astype(np.float32)),
            "bo": bo if s == 0 else np.zeros_like(bo),
        }
        in_maps.append(m)

    global _last_in_maps
    _last_in_maps = in_maps
    res = run_bass_kernel_spmd(nc, in_maps, list(range(8)))
    out = np.empty((B, T, D), dtype=np.float32)
    for b in range(B):
        acc = res.results[2 * b]["outT"] + res.results[2 * b + 1]["outT"]
        out[b] = acc.T
    return out
